# revision 44
# baseline (speedup 1.0000x reference)
"""Single-head attention (B=4, N=4096, H=768, D=64) on 8 TRN2 NeuronCores.

Sharding: core = (batch, query-half). Each core receives the full batch's
x rows (rotated so its 2048 query rows come first -- softmax over keys is
permutation invariant), computes K/V for the whole batch and attention
for its 2048 queries. No collectives: the pair-wise K/V all-gather was
measured ~20-40% fatally flaky on this runtime, so the partner half's
K/V work is duplicated instead. Output reassembled host-side.

Precision: x and the projection weights are cast to bf16 on the host --
bf16 PE transposes and projections run at full rate with fast weight
loads, and halve the input DMA. Scores and attention matmuls run in
float32r (full-rate fp32 streaming). The scores contraction (d=64) is
padded to K=128 with zero rows: fused f32r matmuls with K<128 cannot
overlap their weight load and run ~1.7x slower. K and V share one
projection matmul (stationary [Wk|Wv], M=128). Softmax denominators
come free from a ones column in the V' blocks; exp() is batched over
two PSUM banks to halve ScalarE per-op overhead.
"""

import sys

sys.path.insert(0, "/opt/trn_rl_repo")

import ml_dtypes
import numpy as np

import concourse.tile as tile
from concourse import bacc, mybir
from concourse.bass_utils import run_bass_kernel_spmd
from concourse.masks import make_identity

B = 4
N = 4096          # keys per batch
NQ = 2048         # queries per core
H = 768
D = 64
P = 128
HC = H // P       # 6 contraction chunks
NKB = N // P      # 32 key blocks
NTB = N // 512    # 8 token col-blocks for K/V projections
NQTB = NQ // 512  # 4 token col-blocks for Q
NCORES = 8

DT = mybir.dt.float32
FDT = mybir.dt.float32r
BF = mybir.dt.bfloat16

AF = mybir.ActivationFunctionType


def _attention_head(ctx, tc, out, x, Ws, biases):
    nc = tc.nc
    Wq, Wk, Wv = Ws
    bq, bk, bv = biases

    const = ctx.enter_context(tc.tile_pool(name="const", bufs=1))
    big = ctx.enter_context(tc.tile_pool(name="big", bufs=1))
    xin = ctx.enter_context(tc.tile_pool(name="xin", bufs=3))
    psS = ctx.enter_context(tc.tile_pool(name="psS", bufs=2, space="PSUM"))
    psA = ctx.enter_context(tc.tile_pool(name="psA", bufs=2, space="PSUM"))
    psO = ctx.enter_context(tc.tile_pool(name="psO", bufs=2, space="PSUM"))
    sbE = ctx.enter_context(tc.tile_pool(name="sbE", bufs=8))
    sbo = ctx.enter_context(tc.tile_pool(name="sbo", bufs=4))

    # --- constants (SWDGE queue; keeps the sync queue free for x) ------
    identb = const.tile([P, P], BF)
    make_identity(nc, identb)
    ident = const.tile([P, P], DT)
    make_identity(nc, ident)

    # HAM warm-up: the PE clock gate defaults to 1.2 GHz and needs ~3.4us
    # of activity to release; burn dummy transposes while waiting for x so
    # the first real matmuls run at 2.4 GHz.
    # weights/biases arrive host-packed in their SBUF layouts; scalar
    # HWDGE queue so they land before the first x token block
    w_q = const.tile([P, HC * D], BF)
    nc.scalar.dma_start(w_q[:], Wq)
    w_kv = const.tile([P, HC * P], BF)  # chunk c: [Wk_c | Wv_c]
    nc.scalar.dma_start(w_kv[:], Wk)
    bias_sb = const.tile([P, 4], DT)
    nc.scalar.dma_start(bias_sb[:], bq)
    pt_w = psO.tile([P, P], DT, name="warm", tag="att")
    for _ in range(10):
        nc.tensor.transpose(pt_w[:], ident[:], ident[:])
    nc.vector.tensor_copy(bias_sb[:, 3:4], pt_w[:, 0:1])

    # --- phases A+B interleaved: hardware DMA-transpose of bf16 x (the
    # xbar path; no PE/DVE involvement), K/V projection per 1024-token
    # group as soon as its xT columns land.
    xT = [
        big.tile([P, HC * 512], BF, name=f"xT{tb}") for tb in range(NTB)
    ]
    kv_sb = big.tile([P, N], DT)
    qT = big.tile([P, NQ], FDT)
    nc.vector.memset(qT[D:P, :].bitcast(DT), 0.0)
    kT = big.tile([P, N], FDT)
    nc.vector.memset(kT[D:P, :].bitcast(DT), 0.0)


    def proj_kv(tb):
        s = slice(tb * 512, (tb + 1) * 512)
        ps = psA.tile([P, 512], DT, tag="a", name="pkv")
        for c in range(HC):
            nc.tensor.matmul(
                ps[:],
                w_kv[:, c * P : (c + 1) * P],
                xT[tb][:, c * 512 : (c + 1) * 512],
                start=(c == 0),
                stop=(c == HC - 1),
            )
        nc.vector.tensor_scalar_add(kT[0:D, s], ps[0:D, :], bias_sb[0:D, 1:2])
        nc.vector.tensor_scalar_add(
            kv_sb[D:P, s], ps[D:P, :], bias_sb[D:P, 2:3]
        )

    def proj_q(tb):
        ps = psA.tile([D, 512], DT, tag="a", name="pq")
        for c in range(HC):
            nc.tensor.matmul(
                ps[:],
                w_q[:, c * D : (c + 1) * D],
                xT[tb][:, c * 512 : (c + 1) * 512],
                start=(c == 0),
                stop=(c == HC - 1),
            )
        nc.vector.tensor_scalar_add(
            qT[0:D, tb * 512 : (tb + 1) * 512], ps[:], bias_sb[0:D, 0:1]
        )

    v_sb = big.tile([P, NKB * (D + 1)], FDT)
    nc.gpsimd.memset(
        v_sb[:].bitcast(DT).rearrange("p (k c) -> p k c", c=D + 1)[
            :, :, D : D + 1
        ],
        1.0,
    )

    def vprime(kb):
        pt = psA.tile([P, D], DT, tag="a", name="pv")
        nc.tensor.transpose(
            pt[:], kv_sb[D:P, kb * P : (kb + 1) * P], ident[D:P, D:P]
        )
        nc.vector.tensor_copy(v_sb[:, kb * (D + 1) : kb * (D + 1) + D], pt[:])

    def load_tb(tb):
        # chain the loads so the DMA rings drain tb0 first instead of
        # round-robining all eight transfers to a common late finish
        inst = nc.sync.dma_start(
            out=xT[tb][:], in_=x[tb].rearrange("p c n -> p (c n)")
        )
        tc.chain_iter_dep("xload", inst.ins)

    # --- phase E: attention ---------------------------------------------
    # qb pairs outer so only two PSUM accumulator banks are live; att
    # matmuls lag scores/exp by two key blocks so the PE never stalls on
    # the ScalarE exp. The qp=0 pass is interleaved into the tg loop so
    # exp starts as soon as the first token group is projected.
    scale = float(D) ** -0.5

    def e_state(qp):
        # att accumulators are allocated lazily (first use) so qp1's
        # early score blocks can be emitted before qp0's finish phase
        # without doubling live PSUM accumulator slots
        return {"q0": qp * 1024, "qp": qp, "att": None, "exs": {}}

    def e_block(st, kbs):
        for kb in kbs:
            sc = psS.tile([P, 1024], DT, tag="s", name="sc")
            for i in range(2):
                nc.tensor.matmul(
                    sc[:, i * 512 : (i + 1) * 512],
                    kT[:, kb * P : (kb + 1) * P],
                    qT[:, st["q0"] + i * 512 : st["q0"] + (i + 1) * 512],
                    start=True,
                    stop=True,
                )
            ex = sbE.tile([P, 1024], FDT, name=f"ex{kb}", tag="ex")
            nc.scalar.activation(ex[:], sc[:], AF.Exp, scale=scale)
            st["exs"][kb] = ex
            if kb >= 3:
                if st["att"] is None:
                    st["att"] = [
                        psO.tile(
                            [D + 1, 512], DT,
                            name=f"att{st['qp']}_{i}", tag="att",
                        )
                        for i in range(2)
                    ]
                _att_mms(nc, st["att"], v_sb, st["exs"].pop(kb - 3), kb - 3)

    def e_finish(st, qp):
        for kb in (NKB - 3, NKB - 2, NKB - 1):
            _att_mms(nc, st["att"], v_sb, st["exs"].pop(kb), kb)
        ob = sbo.tile([P, 8 * D], DT, tag="ob", name=f"ob{qp}")
        for i in range(2):
            asb = sbo.tile([D + 1, 512], DT, tag="asb", name=f"asb{qp}_{i}")
            nc.vector.tensor_copy(asb[:], st["att"][i][:])
            for sub in range(4):
                pt = psA.tile([P, D + 1], DT, tag="a", name="pf")
                nc.tensor.transpose(
                    pt[:],
                    asb[:, sub * P : (sub + 1) * P],
                    ident[: D + 1, : D + 1],
                )
                rc = sbo.tile([P, 1], DT, tag="rc", name=f"rc{qp}_{i}_{sub}")
                nc.vector.reciprocal(rc[:], pt[:, D : D + 1])
                j = i * 4 + sub
                nc.vector.tensor_scalar_mul(
                    ob[:, j * D : (j + 1) * D], pt[:, 0:D], rc[:]
                )
        nc.sync.dma_start(
            out[st["q0"] : st["q0"] + 1024, :].rearrange(
                "(s p) d -> p s d", p=P
            ),
            ob[:].rearrange("p (s d) -> p s d", d=D),
        )

    st0 = e_state(0)
    for tg in range(4):
        load_tb(2 * tg)
        load_tb(2 * tg + 1)
        proj_kv(2 * tg)
        proj_kv(2 * tg + 1)
        if tg < 2:
            proj_q(2 * tg)
            proj_q(2 * tg + 1)
        for kb in range(8 * tg, 8 * tg + 8):
            vprime(kb)
        e_block(st0, range(8 * tg, 8 * tg + 8))

    st1 = e_state(1)
    e_block(st1, range(0, 3))
    e_finish(st0, 0)
    e_block(st1, range(3, NKB))
    e_finish(st1, 1)


def _att_mms(nc, att, v_sb, ex, kb):
    for i in range(2):
        nc.tensor.matmul(
            att[i][:],
            v_sb[:, kb * (D + 1) : (kb + 1) * (D + 1)],
            ex[:, i * 512 : (i + 1) * 512],
            start=(kb == 0),
            stop=(kb == NKB - 1),
        )


_NC_CACHE = None


def _build():
    global _NC_CACHE
    if _NC_CACHE is not None:
        return _NC_CACHE
    nc = bacc.Bacc(
        "TRN2",
        target_bir_lowering=False,
        debug=False,
        enable_asserts=True,
        num_devices=NCORES,
    )
    x = nc.dram_tensor("x", [NTB, P, HC, 512], BF, kind="ExternalInput").ap()
    Wq = nc.dram_tensor("Wq", [P, HC * D], BF, kind="ExternalInput").ap()
    bq = nc.dram_tensor("bq", [P, 4], DT, kind="ExternalInput").ap()
    Wk = nc.dram_tensor("Wk", [P, HC * P], BF, kind="ExternalInput").ap()
    bk = nc.dram_tensor("bk", [1], DT, kind="ExternalInput").ap()
    Wv = nc.dram_tensor("Wv", [1], BF, kind="ExternalInput").ap()
    bv = nc.dram_tensor("bv", [1], DT, kind="ExternalInput").ap()
    out = nc.dram_tensor("out", [NQ, D], DT, kind="ExternalOutput").ap()

    from contextlib import ExitStack

    with tile.TileContext(nc) as tc:
        with ExitStack() as ctx:
            _attention_head(ctx, tc, out, x, (Wq, Wk, Wv), (bq, bk, bv))
    nc.compile()
    _NC_CACHE = nc
    return nc


def _make_in_maps(inputs):
    bf16 = ml_dtypes.bfloat16
    x = np.asarray(inputs["x"], dtype=np.float32).astype(bf16)
    wq, wk, wv = (
        np.asarray(inputs[k], dtype=np.float32).astype(bf16)
        for k in ("Wq", "Wk", "Wv")
    )
    # w_q: [p, (c d)]; w_kv: [p, (c [Wk_c | Wv_c])]
    wq_p = np.ascontiguousarray(
        wq.reshape(HC, P, D).transpose(1, 0, 2).reshape(P, HC * D)
    )
    wkv = np.concatenate(
        [wk.reshape(HC, P, D), wv.reshape(HC, P, D)], axis=2
    )  # [c, p, 128]
    wkv_p = np.ascontiguousarray(
        wkv.transpose(1, 0, 2).reshape(P, HC * P)
    )
    bias_p = np.zeros((P, 4), dtype=np.float32)
    bias_p[:D, 0] = np.asarray(inputs["bq"], dtype=np.float32)
    bias_p[:D, 1] = np.asarray(inputs["bk"], dtype=np.float32)
    bias_p[D:, 2] = np.asarray(inputs["bv"], dtype=np.float32)
    small = {
        "Wq": wq_p,
        "Wk": wkv_p,
        "Wv": np.zeros(1, dtype=bf16),
        "bq": bias_p,
        "bk": np.zeros(1, dtype=np.float32),
        "bv": np.zeros(1, dtype=np.float32),
    }
    in_maps = []
    for core in range(NCORES):
        b, h = divmod(core, 2)
        xb = x[b]
        if h == 1:
            xb = np.concatenate([xb[NQ:], xb[:NQ]], axis=0)
        # pre-transposed tg-major chunk layout [4, HC, 128, N/4]: each
        # (tg, c) block is one contiguous 256 KB DMA
        xt = np.ascontiguousarray(
            xb.T.reshape(HC, P, NTB, 512).transpose(2, 1, 0, 3)
        )
        in_maps.append({"x": xt, **small})
    return in_maps


def _run(inputs, trace=False):
    nc = _build()
    res = run_bass_kernel_spmd(
        nc, _make_in_maps(inputs), core_ids=list(range(NCORES)), trace=trace
    )
    out = np.empty((B, N, D), dtype=np.float32)
    for core in range(NCORES):
        b, h = divmod(core, 2)
        out[b, h * NQ : (h + 1) * NQ] = res.results[core]["out"]
    return out, res


def kernel(**inputs):
    out, _ = _run(inputs, trace=False)
    return out


def _install_ntff_hook():
    """Register the axon NTFF profiling hook that this image's antenv lacks."""
    import types

    try:
        import antenv.axon_hooks  # noqa: F401

        return
    except ImportError:
        pass
    import antenv
    from trn_agent_boot.trn_boot import _ntff_profile_via_ctypes

    import concourse.bass_utils as bu

    mod = types.ModuleType("antenv.axon_hooks")
    _h = [None]
    mod.set_axon_ntff_profile_hook = lambda h: _h.__setitem__(0, h)
    mod.get_axon_ntff_profile_hook = lambda: _h[0]
    sys.modules["antenv.axon_hooks"] = mod
    antenv.axon_hooks = mod
    mod.set_axon_ntff_profile_hook(
        _ntff_profile_via_ctypes("/opt/axon/libaxon_pjrt.so")
    )
    bu.upload_artifacts = lambda tmpdir: tmpdir


def run_traced(inputs):
    _install_ntff_hook()
    out, res = _run(inputs, trace=True)
    return out, res.exec_time_ns


# revision 45
# speedup vs baseline: 1.1665x; 1.1665x over previous
"""Single-head attention (B=4, N=4096, H=768, D=64) on 8 TRN2 NeuronCores.

Sharding: core = (batch, query-half). Each core receives the full batch's
x rows (rotated so its 2048 query rows come first -- softmax over keys is
permutation invariant), computes K/V for the whole batch and attention
for its 2048 queries. No collectives: the pair-wise K/V all-gather was
measured ~20-40% fatally flaky on this runtime, so the partner half's
K/V work is duplicated instead. Output reassembled host-side.

Precision: x and the projection weights are cast to bf16 on the host --
bf16 PE transposes and projections run at full rate with fast weight
loads, and halve the input DMA. Scores and attention matmuls run in
float32r (full-rate fp32 streaming). The scores contraction (d=64) is
padded to K=128 with zero rows: fused f32r matmuls with K<128 cannot
overlap their weight load and run ~1.7x slower. K and V share one
projection matmul (stationary [Wk|Wv], M=128). Softmax denominators
come free from a ones column in the V' blocks; exp() is batched over
two PSUM banks to halve ScalarE per-op overhead.
"""

import sys

sys.path.insert(0, "/opt/trn_rl_repo")

import ml_dtypes
import numpy as np

import concourse.tile as tile
from concourse import bacc, mybir
from concourse.bass_utils import run_bass_kernel_spmd
from concourse.masks import make_identity

B = 4
N = 4096          # keys per batch
NQ = 2048         # queries per core
H = 768
D = 64
P = 128
HC = H // P       # 6 contraction chunks
NKB = N // P      # 32 key blocks
NTB = N // 512    # 8 token col-blocks for K/V projections
NQTB = NQ // 512  # 4 token col-blocks for Q
NCORES = 8

DT = mybir.dt.float32
FDT = mybir.dt.float32r
BF = mybir.dt.bfloat16

AF = mybir.ActivationFunctionType


def _attention_head(ctx, tc, out, x, Ws, biases):
    nc = tc.nc
    Wq, Wk, Wv = Ws
    bq, bk, bv = biases

    const = ctx.enter_context(tc.tile_pool(name="const", bufs=1))
    big = ctx.enter_context(tc.tile_pool(name="big", bufs=1))
    xin = ctx.enter_context(tc.tile_pool(name="xin", bufs=3))
    psS = ctx.enter_context(tc.tile_pool(name="psS", bufs=2, space="PSUM"))
    psA = ctx.enter_context(tc.tile_pool(name="psA", bufs=2, space="PSUM"))
    psO = ctx.enter_context(tc.tile_pool(name="psO", bufs=2, space="PSUM"))
    sbE = ctx.enter_context(tc.tile_pool(name="sbE", bufs=8))
    sbo = ctx.enter_context(tc.tile_pool(name="sbo", bufs=4))

    # --- constants (SWDGE queue; keeps the sync queue free for x) ------
    identb = const.tile([P, P], BF)
    make_identity(nc, identb)
    ident = const.tile([P, P], DT)
    make_identity(nc, ident)

    # HAM warm-up: the PE clock gate defaults to 1.2 GHz and needs ~3.4us
    # of activity to release; burn dummy transposes while waiting for x so
    # the first real matmuls run at 2.4 GHz.
    # weights/biases arrive host-packed in their SBUF layouts; scalar
    # HWDGE queue so they land before the first x token block
    w_q = const.tile([P, HC * D], BF)
    nc.scalar.dma_start(w_q[:], Wq)
    w_kv = const.tile([P, HC * P], BF)  # chunk c: [Wk_c | Wv_c]
    nc.scalar.dma_start(w_kv[:], Wk)
    bias_sb = const.tile([P, 4], DT)
    nc.scalar.dma_start(bias_sb[:], bq)
    pt_w = psO.tile([P, P], DT, name="warm", tag="att")
    for _ in range(24):
        nc.tensor.transpose(pt_w[:], ident[:], ident[:])
    nc.vector.tensor_copy(bias_sb[:, 3:4], pt_w[:, 0:1])

    # --- phases A+B interleaved: hardware DMA-transpose of bf16 x (the
    # xbar path; no PE/DVE involvement), K/V projection per 1024-token
    # group as soon as its xT columns land.
    xT = [
        big.tile([P, HC * 512], BF, name=f"xT{tb}") for tb in range(NTB)
    ]
    kv_sb = big.tile([P, N], DT)
    qT = big.tile([P, NQ], FDT)
    nc.vector.memset(qT[D:P, :].bitcast(DT), 0.0)
    kT = big.tile([P, N], FDT)
    nc.vector.memset(kT[D:P, :].bitcast(DT), 0.0)


    def proj_kv(tb):
        s = slice(tb * 512, (tb + 1) * 512)
        ps = psA.tile([P, 512], DT, tag="a", name="pkv")
        for c in range(HC):
            nc.tensor.matmul(
                ps[:],
                w_kv[:, c * P : (c + 1) * P],
                xT[tb][:, c * 512 : (c + 1) * 512],
                start=(c == 0),
                stop=(c == HC - 1),
            )
        nc.vector.tensor_scalar_add(kT[0:D, s], ps[0:D, :], bias_sb[0:D, 1:2])
        nc.vector.tensor_scalar_add(
            kv_sb[D:P, s], ps[D:P, :], bias_sb[D:P, 2:3]
        )

    def proj_q(tb):
        ps = psA.tile([D, 512], DT, tag="a", name="pq")
        for c in range(HC):
            nc.tensor.matmul(
                ps[:],
                w_q[:, c * D : (c + 1) * D],
                xT[tb][:, c * 512 : (c + 1) * 512],
                start=(c == 0),
                stop=(c == HC - 1),
            )
        nc.vector.tensor_scalar_add(
            qT[0:D, tb * 512 : (tb + 1) * 512], ps[:], bias_sb[0:D, 0:1]
        )

    v_sb = big.tile([P, NKB * (D + 1)], FDT)
    nc.gpsimd.memset(
        v_sb[:].bitcast(DT).rearrange("p (k c) -> p k c", c=D + 1)[
            :, :, D : D + 1
        ],
        1.0,
    )

    def vprime(kb):
        pt = psA.tile([P, D], DT, tag="a", name="pv")
        nc.tensor.transpose(
            pt[:], kv_sb[D:P, kb * P : (kb + 1) * P], ident[D:P, D:P]
        )
        nc.vector.tensor_copy(v_sb[:, kb * (D + 1) : kb * (D + 1) + D], pt[:])

    def load_tb(tb):
        # chain the loads so the DMA rings drain tb0 first instead of
        # round-robining all eight transfers to a common late finish
        inst = nc.sync.dma_start(
            out=xT[tb][:], in_=x[tb].rearrange("p c n -> p (c n)")
        )
        tc.chain_iter_dep("xload", inst.ins)

    # --- phase E: attention ---------------------------------------------
    # qb pairs outer so only two PSUM accumulator banks are live; att
    # matmuls lag scores/exp by two key blocks so the PE never stalls on
    # the ScalarE exp. The qp=0 pass is interleaved into the tg loop so
    # exp starts as soon as the first token group is projected.
    scale = float(D) ** -0.5

    def e_state(qp):
        # att accumulators are allocated lazily (first use) so qp1's
        # early score blocks can be emitted before qp0's finish phase
        # without doubling live PSUM accumulator slots
        return {"q0": qp * 1024, "qp": qp, "att": None, "exs": {}}

    def e_block(st, kbs):
        for kb in kbs:
            sc = psS.tile([P, 1024], DT, tag="s", name="sc")
            for i in range(2):
                nc.tensor.matmul(
                    sc[:, i * 512 : (i + 1) * 512],
                    kT[:, kb * P : (kb + 1) * P],
                    qT[:, st["q0"] + i * 512 : st["q0"] + (i + 1) * 512],
                    start=True,
                    stop=True,
                )
            ex = sbE.tile([P, 1024], FDT, name=f"ex{kb}", tag="ex")
            nc.scalar.activation(ex[:], sc[:], AF.Exp, scale=scale)
            st["exs"][kb] = ex
            if kb >= 3:
                if st["att"] is None:
                    st["att"] = [
                        psO.tile(
                            [D + 1, 512], DT,
                            name=f"att{st['qp']}_{i}", tag="att",
                        )
                        for i in range(2)
                    ]
                _att_mms(nc, st["att"], v_sb, st["exs"].pop(kb - 3), kb - 3)

    def e_finish(st, qp):
        for kb in (NKB - 3, NKB - 2, NKB - 1):
            _att_mms(nc, st["att"], v_sb, st["exs"].pop(kb), kb)
        ob = sbo.tile([P, 8 * D], DT, tag="ob", name=f"ob{qp}")
        for i in range(2):
            asb = sbo.tile([D + 1, 512], DT, tag="asb", name=f"asb{qp}_{i}")
            nc.vector.tensor_copy(asb[:], st["att"][i][:])
            for sub in range(4):
                pt = psA.tile([P, D + 1], DT, tag="a", name="pf")
                nc.tensor.transpose(
                    pt[:],
                    asb[:, sub * P : (sub + 1) * P],
                    ident[: D + 1, : D + 1],
                )
                rc = sbo.tile([P, 1], DT, tag="rc", name=f"rc{qp}_{i}_{sub}")
                nc.vector.reciprocal(rc[:], pt[:, D : D + 1])
                j = i * 4 + sub
                nc.vector.tensor_scalar_mul(
                    ob[:, j * D : (j + 1) * D], pt[:, 0:D], rc[:]
                )
        nc.sync.dma_start(
            out[st["q0"] : st["q0"] + 1024, :].rearrange(
                "(s p) d -> p s d", p=P
            ),
            ob[:].rearrange("p (s d) -> p s d", d=D),
        )

    st0 = e_state(0)
    for tg in range(4):
        load_tb(2 * tg)
        load_tb(2 * tg + 1)
        proj_kv(2 * tg)
        proj_kv(2 * tg + 1)
        if tg < 2:
            proj_q(2 * tg)
            proj_q(2 * tg + 1)
        for kb in range(8 * tg, 8 * tg + 8):
            vprime(kb)
        e_block(st0, range(8 * tg, 8 * tg + 8))

    st1 = e_state(1)
    e_block(st1, range(0, 3))
    e_finish(st0, 0)
    e_block(st1, range(3, NKB))
    e_finish(st1, 1)


def _att_mms(nc, att, v_sb, ex, kb):
    for i in range(2):
        nc.tensor.matmul(
            att[i][:],
            v_sb[:, kb * (D + 1) : (kb + 1) * (D + 1)],
            ex[:, i * 512 : (i + 1) * 512],
            start=(kb == 0),
            stop=(kb == NKB - 1),
        )


_NC_CACHE = None


def _build():
    global _NC_CACHE
    if _NC_CACHE is not None:
        return _NC_CACHE
    nc = bacc.Bacc(
        "TRN2",
        target_bir_lowering=False,
        debug=False,
        enable_asserts=True,
        num_devices=NCORES,
    )
    x = nc.dram_tensor("x", [NTB, P, HC, 512], BF, kind="ExternalInput").ap()
    Wq = nc.dram_tensor("Wq", [P, HC * D], BF, kind="ExternalInput").ap()
    bq = nc.dram_tensor("bq", [P, 4], DT, kind="ExternalInput").ap()
    Wk = nc.dram_tensor("Wk", [P, HC * P], BF, kind="ExternalInput").ap()
    bk = nc.dram_tensor("bk", [1], DT, kind="ExternalInput").ap()
    Wv = nc.dram_tensor("Wv", [1], BF, kind="ExternalInput").ap()
    bv = nc.dram_tensor("bv", [1], DT, kind="ExternalInput").ap()
    out = nc.dram_tensor("out", [NQ, D], DT, kind="ExternalOutput").ap()

    from contextlib import ExitStack

    with tile.TileContext(nc) as tc:
        with ExitStack() as ctx:
            _attention_head(ctx, tc, out, x, (Wq, Wk, Wv), (bq, bk, bv))
    nc.compile()
    _NC_CACHE = nc
    return nc


def _make_in_maps(inputs):
    bf16 = ml_dtypes.bfloat16
    x = np.asarray(inputs["x"], dtype=np.float32).astype(bf16)
    wq, wk, wv = (
        np.asarray(inputs[k], dtype=np.float32).astype(bf16)
        for k in ("Wq", "Wk", "Wv")
    )
    # w_q: [p, (c d)]; w_kv: [p, (c [Wk_c | Wv_c])]
    wq_p = np.ascontiguousarray(
        wq.reshape(HC, P, D).transpose(1, 0, 2).reshape(P, HC * D)
    )
    wkv = np.concatenate(
        [wk.reshape(HC, P, D), wv.reshape(HC, P, D)], axis=2
    )  # [c, p, 128]
    wkv_p = np.ascontiguousarray(
        wkv.transpose(1, 0, 2).reshape(P, HC * P)
    )
    bias_p = np.zeros((P, 4), dtype=np.float32)
    bias_p[:D, 0] = np.asarray(inputs["bq"], dtype=np.float32)
    bias_p[:D, 1] = np.asarray(inputs["bk"], dtype=np.float32)
    bias_p[D:, 2] = np.asarray(inputs["bv"], dtype=np.float32)
    small = {
        "Wq": wq_p,
        "Wk": wkv_p,
        "Wv": np.zeros(1, dtype=bf16),
        "bq": bias_p,
        "bk": np.zeros(1, dtype=np.float32),
        "bv": np.zeros(1, dtype=np.float32),
    }
    in_maps = []
    for core in range(NCORES):
        b, h = divmod(core, 2)
        xb = x[b]
        if h == 1:
            xb = np.concatenate([xb[NQ:], xb[:NQ]], axis=0)
        # pre-transposed tg-major chunk layout [4, HC, 128, N/4]: each
        # (tg, c) block is one contiguous 256 KB DMA
        xt = np.ascontiguousarray(
            xb.T.reshape(HC, P, NTB, 512).transpose(2, 1, 0, 3)
        )
        in_maps.append({"x": xt, **small})
    return in_maps


def _run(inputs, trace=False):
    nc = _build()
    res = run_bass_kernel_spmd(
        nc, _make_in_maps(inputs), core_ids=list(range(NCORES)), trace=trace
    )
    out = np.empty((B, N, D), dtype=np.float32)
    for core in range(NCORES):
        b, h = divmod(core, 2)
        out[b, h * NQ : (h + 1) * NQ] = res.results[core]["out"]
    return out, res


def kernel(**inputs):
    out, _ = _run(inputs, trace=False)
    return out


def _install_ntff_hook():
    """Register the axon NTFF profiling hook that this image's antenv lacks."""
    import types

    try:
        import antenv.axon_hooks  # noqa: F401

        return
    except ImportError:
        pass
    import antenv
    from trn_agent_boot.trn_boot import _ntff_profile_via_ctypes

    import concourse.bass_utils as bu

    mod = types.ModuleType("antenv.axon_hooks")
    _h = [None]
    mod.set_axon_ntff_profile_hook = lambda h: _h.__setitem__(0, h)
    mod.get_axon_ntff_profile_hook = lambda: _h[0]
    sys.modules["antenv.axon_hooks"] = mod
    antenv.axon_hooks = mod
    mod.set_axon_ntff_profile_hook(
        _ntff_profile_via_ctypes("/opt/axon/libaxon_pjrt.so")
    )
    bu.upload_artifacts = lambda tmpdir: tmpdir


def run_traced(inputs):
    _install_ntff_hook()
    out, res = _run(inputs, trace=True)
    return out, res.exec_time_ns
